# revision 16
# baseline (speedup 1.0000x reference)
"""GRU+attention decoder step on 8 TRN2 NeuronCores.

Strategy (hardcoded for B=64, S=1024, H=E=1024, V=32000, 8 cores):
  - Attention: data-parallel over batch (8 batches/core). Shift-free softmax:
    energy>=0 and tiny, so exp() accumulated directly; context accumulated in
    PSUM across s-tiles via col-group (tile_position) matmuls; normalized by
    1/z at extraction.
  - AllGather x=[context,embedded] -> GRU sharded over H (128 h-cols/core),
    weights host-pretransposed so PE needs no on-device weight transposes.
  - AllGather h_newT -> output projection sharded over vocab (4000 cols/core).
  - Biases applied via K=1 matmuls (ones row) straight into PSUM.
  - walrus embeds at most ONE sync wait per instruction, so tiny single-input
    "absorber" ops (DVE/ACT copies, 1x1 PE matmuls) pre-sync each engine's
    vector clock before any instruction that would otherwise need 2+ waits.
Host pre-layout = slicing/transposing input arrays only; all model math
(embedding gather, tanh/energy/softmax/context, GRU, 64x32000 projection)
runs on device.
"""

import os
import sys

import numpy as np

try:
    import concourse.bass as bass  # noqa: F401
except Exception:
    sys.path.insert(0, "/opt/trn_rl_repo")

V, E, H = 32000, 1024, 1024
B, S = 64, 1024
NC = 8
BL = B // NC      # 8 batches per core
HL = H // NC      # 128 h columns per core
VL = V // NC      # 4000 vocab per core
ST = S // 128     # 8 s-tiles
NVC = 8
VC = VL // NVC    # 500 vocab per psum chunk

_NC_CACHE = {}
LAST_RESULTS = None


def _build():
    from contextlib import ExitStack

    import concourse.bass as bass
    import concourse.tile as tile
    from concourse import bacc, mybir
    from concourse.masks import make_identity

    f32 = mybir.dt.float32
    f32r = mybir.dt.float32r
    i32 = mybir.dt.int32
    AF = mybir.ActivationFunctionType
    ALU = mybir.AluOpType

    nc = bacc.Bacc("TRN2", num_devices=NC, name="gru_attn_dec")

    enc = nc.dram_tensor("enc", [S, BL, H], f32, kind="ExternalInput")
    dhsl = nc.dram_tensor("dhsl", [BL, H], f32, kind="ExternalInput")
    dhT = nc.dram_tensor("dhT", [H, B], f32r, kind="ExternalInput")
    dhhs = nc.dram_tensor("dhhs", [B, HL], f32, kind="ExternalInput")
    idx = nc.dram_tensor("idx", [BL, 1], i32, kind="ExternalInput")
    emb = nc.dram_tensor("emb", [V, E], f32r, kind="ExternalInput")
    wen = nc.dram_tensor("wen", [2, H], f32, kind="ExternalInput")
    ben = nc.dram_tensor("ben", [1, 1], f32, kind="ExternalInput")
    wiht = nc.dram_tensor("wiht", [2 * E, 3 * HL], f32r, kind="ExternalInput")
    whht = nc.dram_tensor("whht", [H, 3 * HL], f32r, kind="ExternalInput")
    bih = nc.dram_tensor("bih", [1, 3 * HL], f32r, kind="ExternalInput")
    bhh = nc.dram_tensor("bhh", [1, 3 * HL], f32r, kind="ExternalInput")
    woutt = nc.dram_tensor("woutt", [H, VL], f32r, kind="ExternalInput")
    onesd = nc.dram_tensor("onesd", [1, B], f32, kind="ExternalInput")
    onesr = nc.dram_tensor("onesr", [1, B], f32r, kind="ExternalInput")
    identr = nc.dram_tensor("identr", [128, 128], f32r, kind="ExternalInput")
    bout = nc.dram_tensor("bout", [1, VL], f32r, kind="ExternalInput")

    out_l = nc.dram_tensor("out_l", [B, VL], f32, kind="ExternalOutput")
    hn_o = nc.dram_tensor("hn_o", [B, HL], f32, kind="ExternalOutput")

    rg = [list(range(NC))]

    with tile.TileContext(nc) as tc, ExitStack() as top:
        dram = top.enter_context(tc.tile_pool(name="dram", bufs=1, space="DRAM"))
        consts = top.enter_context(tc.tile_pool(name="consts", bufs=1))
        wout_pool = top.enter_context(tc.tile_pool(name="woutp", bufs=3))

        x_loc = dram.tile([BL, 2 * E], f32r, name="x_loc")
        x_full = dram.tile([B, 2 * E], f32r, name="x_full")
        hT_loc = dram.tile([HL, B], f32r, name="hT_loc")
        hT_full = dram.tile([H, B], f32r, name="hT_full")
        c_d = dram.tile([1, BL], f32, name="c_d")
        z_d = dram.tile([1, BL], f32, name="z_d")

        # --- constants / one-time prep ---
        we_b = consts.tile([128, H], f32, name="we_b")
        nc.gpsimd.dma_start(out=we_b, in_=wen[1:2, :].to_broadcast([128, H]))
        wh_b = consts.tile([BL, H], f32, name="wh_b")
        nc.gpsimd.dma_start(out=wh_b, in_=wen[0:1, :].to_broadcast([BL, H]))
        ben_b = consts.tile([BL, 1], f32, name="ben_b")
        nc.gpsimd.dma_start(out=ben_b, in_=ben[0:1, :].to_broadcast([BL, 1]))

        ident_f = consts.tile([128, 128], f32, name="ident_f")
        make_identity(nc, ident_f)

        ones_col = consts.tile([128, 1], f32, name="ones_col")
        nc.gpsimd.dma_start(out=ones_col, in_=onesd[0:1, 0:1].to_broadcast([128, 1]))
        ones_row = consts.tile([1, B], f32r, name="ones_row")
        nc.sync.dma_start(out=ones_row, in_=onesr[:, :])
        identr_sb = consts.tile([128, 128], f32r, name="identr_sb")
        nc.sync.dma_start(out=identr_sb, in_=identr[:, :])

        # tiny scratch for wait-absorbers
        dsc = consts.tile([128, 8], f32, name="dsc")

        idx_t = consts.tile([BL, 1], i32, name="idx_t")
        nc.sync.dma_start(out=idx_t, in_=idx[:, :])
        embt = consts.tile([BL, E], f32r, name="embt")
        nc.gpsimd.indirect_dma_start(
            out=embt[:, :],
            out_offset=None,
            in_=emb[:, :],
            in_offset=bass.IndirectOffsetOnAxis(ap=idx_t[:, 0:1], axis=0),
        )

        # th = tanh(decoder_hidden[b_slice]); c_b = th.w_h + b_energy
        dh_t = consts.tile([BL, H], f32, name="dh_t")
        nc.sync.dma_start(out=dh_t, in_=dhsl[:, :])
        th_t = consts.tile([BL, H], f32, name="th_t")
        nc.scalar.activation(out=th_t, in_=dh_t, func=AF.Tanh)
        cgarb = consts.tile([BL, H], f32, name="cgarb")
        c_col = consts.tile([BL, 1], f32, name="c_col")
        # absorber: DVE syncs wh_b's DMA before the 2-input op below
        nc.vector.tensor_copy(dsc[0:BL, 0:1], wh_b[:, 0:1])
        nc.vector.scalar_tensor_tensor(
            out=cgarb,
            in0=th_t,
            scalar=1.0,
            in1=wh_b,
            op0=ALU.mult,
            op1=ALU.mult,
            accum_out=c_col[:, 0:1],
        )
        # absorber: sync ben_b's DMA
        nc.vector.tensor_copy(dsc[0:BL, 1:2], ben_b[:, 0:1])
        c_col2 = consts.tile([BL, 1], f32, name="c_col2")
        nc.vector.tensor_add(c_col2, c_col, ben_b)
        nc.sync.dma_start(out=c_d[0:1, :], in_=c_col2[:, 0:1])
        c128 = consts.tile([128, BL], f32, name="c128")
        nc.gpsimd.dma_start(out=c128, in_=c_d[0:1, :].to_broadcast([128, BL]))

        # --- attention over s-tiles ---
        with ExitStack() as actx:
            enc_p = actx.enter_context(tc.tile_pool(name="encp", bufs=2))
            tanh_p = actx.enter_context(tc.tile_pool(name="tanhp", bufs=2))
            small_p = actx.enter_context(tc.tile_pool(name="smallp", bufs=2))
            apsum = actx.enter_context(
                tc.tile_pool(name="apsum", bufs=1, space="PSUM")
            )
            garb_ps = apsum.tile([128, H], f32, name="garb")
            ctxA = apsum.tile([128, H], f32, name="ctxA")
            ctxB = apsum.tile([128, H], f32, name="ctxB")
            z_ps = apsum.tile([1, BL], f32, name="z_ps")
            dps = apsum.tile([1, BL], f32, name="dps")

            p_spread = consts.tile([128, 8 * 32], f32, name="p_spread")
            nc.vector.memset(p_spread, 0.0)
            # PE absorbers: sync DVE (covers memsets) and gpsimd (identities)
            nc.tensor.matmul(dps[0:1, 0:1], p_spread[:, 0:1], p_spread[:, 0:1])
            nc.tensor.matmul(dps[0:1, 1:2], ident_f[:, 0:1], ident_f[:, 0:1])
            # DVE absorbers for broadcast DMAs consumed inside the loop
            nc.vector.tensor_copy(dsc[:, 2:3], we_b[:, 0:1])
            nc.vector.tensor_copy(dsc[:, 3:4], c128[:, 0:1])

            tch_a = consts.tile([1, 8], f32, name="tch_a")

            for si in range(ST):
                enc_t = enc_p.tile([128, BL, H], f32, name="enc_t")
                nc.sync.dma_start(out=enc_t, in_=enc[si * 128:(si + 1) * 128, :, :])
                # ACT absorber: sync enc DMA so tanh ops need only 1 wait
                nc.scalar.copy(tch_a[0:1, 0:1], enc_t[0:1, 0, 0:1])
                e_t = small_p.tile([128, BL], f32, name="e_t")
                for q in range(4):
                    tq = tanh_p.tile([128, 2, H], f32, name="tq")
                    nc.scalar.activation(
                        out=tq, in_=enc_t[:, 2 * q:2 * q + 2, :], func=AF.Tanh
                    )
                    for j in range(2):
                        b = 2 * q + j
                        nc.vector.scalar_tensor_tensor(
                            out=garb_ps,
                            in0=tq[:, j, :],
                            scalar=1.0,
                            in1=we_b,
                            op0=ALU.mult,
                            op1=ALU.mult,
                            accum_out=e_t[:, b:b + 1],
                        )
                e2_t = small_p.tile([128, BL], f32, name="e2_t")
                nc.vector.tensor_add(e2_t, e_t, c128)
                er_t = small_p.tile([128, BL], f32, name="er_t")
                nc.vector.tensor_scalar_max(er_t, e2_t, 0.0)
                # ACT absorber: sync PE's last read of p_spread (WAR) before exp
                if si == 0:
                    nc.scalar.copy(tch_a[0:1, 1:2], dps[0:1, 0:1])
                else:
                    nc.scalar.copy(tch_a[0:1, 1:2], ctxB[96:97, 0:1])
                nc.scalar.activation(out=p_spread[:, ::32], in_=er_t, func=AF.Exp)

                nc.tensor.matmul(
                    z_ps,
                    ones_col,
                    p_spread[:, ::32],
                    start=(si == 0),
                    stop=(si == ST - 1),
                )
                for b in range(BL):
                    jj = b % 4
                    ctx_ps = ctxA if b < 4 else ctxB
                    for c in range(2):
                        nc.tensor.matmul(
                            ctx_ps[32 * jj:32 * jj + 32, c * 512:(c + 1) * 512],
                            p_spread[:, b * 32:(b + 1) * 32],
                            enc_t[:, b, c * 512:(c + 1) * 512],
                            start=(si == 0),
                            stop=(si == ST - 1),
                            tile_position=(0, 32 * jj),
                        )

            z_sb = small_p.tile([1, BL], f32, name="z_sb")
            nc.vector.reciprocal(z_sb, z_ps)
            nc.sync.dma_start(out=z_d[0:1, :], in_=z_sb)
            zi128 = small_p.tile([128, BL], f32, name="zi128")
            nc.gpsimd.dma_start(out=zi128, in_=z_d[0:1, :].to_broadcast([128, BL]))
            # DVE absorber: sync zi128 DMA before psum-reading scaled copies
            nc.vector.tensor_copy(dsc[:, 4:5], zi128[:, 0:1])

            stgA = small_p.tile([128, H], f32r, name="stgA")
            stgB = small_p.tile([128, H], f32r, name="stgB")
            for b in range(BL):
                jj = b % 4
                src, stg = (ctxA, stgA) if b < 4 else (ctxB, stgB)
                nc.vector.tensor_scalar_mul(
                    stg[32 * jj:32 * jj + 1, :],
                    src[32 * jj:32 * jj + 1, :],
                    zi128[32 * jj:32 * jj + 1, b:b + 1],
                )
            nc.sync.dma_start(out=x_loc[0:4, 0:E], in_=stgA[::32, :])
            nc.sync.dma_start(out=x_loc[4:8, 0:E], in_=stgB[::32, :])
            nc.sync.dma_start(out=x_loc[:, E:2 * E], in_=embt)

        nc.gpsimd.collective_compute(
            "AllGather",
            ALU.bypass,
            replica_groups=rg,
            ins=[x_loc[:, :].opt()],
            outs=[x_full[:, :].opt()],
        )

        # --- GRU (H-sharded: this core computes h columns k*128..k*128+127) ---
        with ExitStack() as gctx:
            gp = gctx.enter_context(tc.tile_pool(name="gp", bufs=1))
            gps = gctx.enter_context(tc.tile_pool(name="gpsum", bufs=1, space="PSUM"))
            gps2 = gctx.enter_context(
                tc.tile_pool(name="gpsum2", bufs=2, space="PSUM")
            )
            gdum = gps.tile([1, BL], f32, name="gdum")

            x_sb = gp.tile([B, 2 * E], f32r, name="x_sb")
            nc.sync.dma_start(out=x_sb, in_=x_full[:, :])
            xT = gp.tile([128, 16, B], f32r, name="xT")
            for kt in range(16):
                tp = gps2.tile([128, B], f32r, name="tp")
                nc.tensor.transpose(
                    tp, x_sb[:, kt * 128:(kt + 1) * 128], identr_sb[0:B, 0:B]
                )
                nc.vector.tensor_copy(xT[:, kt, :], tp)

            wih_sb = gp.tile([128, 16, 3 * HL], f32r, name="wih_sb")
            nc.sync.dma_start(out=wih_sb, in_=wiht.rearrange("(t p) g -> p t g", p=128))
            whh_sb = gp.tile([128, 8, 3 * HL], f32r, name="whh_sb")
            nc.sync.dma_start(out=whh_sb, in_=whht.rearrange("(t p) g -> p t g", p=128))
            dhT_sb = gp.tile([128, 8, B], f32r, name="dhT_sb")
            nc.sync.dma_start(out=dhT_sb, in_=dhT.rearrange("(t p) b -> p t b", p=128))
            bih_sb = gp.tile([1, 3 * HL], f32r, name="bih_sb")
            nc.sync.dma_start(out=bih_sb, in_=bih[:, :])
            bhh_sb = gp.tile([1, 3 * HL], f32r, name="bhh_sb")
            nc.sync.dma_start(out=bhh_sb, in_=bhh[:, :])

            # PE absorbers: one per weight-load DMA lane
            nc.tensor.matmul(gdum[0:1, 0:1], wih_sb[:, 0, 0:1].bitcast(f32), wih_sb[:, 0, 0:1].bitcast(f32))
            nc.tensor.matmul(gdum[0:1, 1:2], whh_sb[:, 0, 0:1].bitcast(f32), whh_sb[:, 0, 0:1].bitcast(f32))
            nc.tensor.matmul(gdum[0:1, 2:3], dhT_sb[:, 0, 0:1].bitcast(f32), dhT_sb[:, 0, 0:1].bitcast(f32))

            gi = gps.tile([B, 3 * HL], f32, name="gi")
            nc.tensor.matmul(gi, ones_row, bih_sb, start=True, stop=False)
            for kt in range(16):
                nc.tensor.matmul(
                    gi, xT[:, kt, :], wih_sb[:, kt, :], start=False, stop=(kt == 15)
                )
            gh = gps.tile([B, 3 * HL], f32, name="gh")
            nc.tensor.matmul(gh, ones_row, bhh_sb, start=True, stop=False)
            for kt in range(8):
                nc.tensor.matmul(
                    gh, dhT_sb[:, kt, :], whh_sb[:, kt, :], start=False, stop=(kt == 7)
                )

            # gates: r,z = sigmoid = 0.5*tanh(0.5x)+0.5 ; n = tanh(i_n + r*h_n)
            gh_s = gp.tile([B, 3 * HL], f32, name="gh_s")
            nc.vector.tensor_copy(gh_s, gh)
            rz_s = gp.tile([B, 2 * HL], f32, name="rz_s")
            nc.vector.tensor_add(rz_s, gi[:, 0:2 * HL], gh_s[:, 0:2 * HL])
            rz_t = gp.tile([B, 2 * HL], f32, name="rz_t")
            nc.scalar.activation(out=rz_t, in_=rz_s, func=AF.Tanh, scale=0.5)
            rz = gp.tile([B, 2 * HL], f32, name="rz")
            nc.vector.tensor_scalar(rz, rz_t, 0.5, 0.5, ALU.mult, ALU.add)

            tmpn = gp.tile([B, HL], f32, name="tmpn")
            nc.vector.tensor_mul(tmpn, rz[:, 0:HL], gh_s[:, 2 * HL:3 * HL])
            npre = gp.tile([B, HL], f32, name="npre")
            nc.vector.tensor_add(npre, tmpn, gi[:, 2 * HL:3 * HL])
            n_t = gp.tile([B, HL], f32, name="n_t")
            nc.scalar.activation(out=n_t, in_=npre, func=AF.Tanh)

            hhs = gp.tile([B, HL], f32, name="hhs")
            nc.sync.dma_start(out=hhs, in_=dhhs[:, :])
            dif = gp.tile([B, HL], f32, name="dif")
            nc.vector.tensor_sub(dif, hhs, n_t)
            zd = gp.tile([B, HL], f32, name="zd")
            nc.vector.tensor_mul(zd, rz[:, HL:2 * HL], dif)
            hnew = gp.tile([B, HL], f32, name="hnew")
            nc.vector.tensor_add(hnew, n_t, zd)
            nc.sync.dma_start(out=hn_o[:, :], in_=hnew)

            tp2 = gps2.tile([128, B], f32, name="tp2")
            nc.tensor.transpose(tp2, hnew, ident_f[0:B, 0:B])
            hT_sb = gp.tile([128, B], f32r, name="hT_sb")
            nc.vector.tensor_copy(hT_sb, tp2)
            nc.sync.dma_start(out=hT_loc[:, :], in_=hT_sb)

        nc.gpsimd.collective_compute(
            "AllGather",
            ALU.bypass,
            replica_groups=rg,
            ins=[hT_loc[:, :].opt()],
            outs=[hT_full[:, :].opt()],
        )

        # --- output projection (vocab-sharded) ---
        with ExitStack() as pctx:
            pp = pctx.enter_context(tc.tile_pool(name="pp", bufs=1))
            pps = pctx.enter_context(tc.tile_pool(name="ppsum", bufs=1, space="PSUM"))
            hTf_sb = pp.tile([128, 8, B], f32r, name="hTf_sb")
            nc.sync.dma_start(
                out=hTf_sb, in_=hT_full.rearrange("(t p) b -> p t b", p=128)
            )
            bout_sb = pp.tile([1, VL], f32r, name="bout_sb")
            nc.sync.dma_start(out=bout_sb, in_=bout[:, :])
            out_sb = pp.tile([B, VL], f32, name="out_sb")

            ops = [
                pps.tile([B, VC], f32, name=f"ops{v}", tag=f"ops{v}")
                for v in range(NVC)
            ]
            # PE absorber for hTf_sb's DMA (overwritten by the bias matmul)
            nc.tensor.matmul(
                ops[0][0:1, 0:1], hTf_sb[:, 0, 0:1].bitcast(f32), hTf_sb[:, 0, 0:1].bitcast(f32),
                start=True, stop=True, skip_group_check=True,
            )
            for vc in range(NVC):
                nc.tensor.matmul(
                    ops[vc],
                    ones_row,
                    bout_sb[:, vc * VC:(vc + 1) * VC],
                    start=True,
                    stop=False,
                    skip_group_check=True,
                )
            for t in range(8):
                slab = wout_pool.tile([128, VL], f32r, name="slab")
                nc.sync.dma_start(out=slab, in_=woutt[t * 128:(t + 1) * 128, :])
                for vc in range(NVC):
                    nc.tensor.matmul(
                        ops[vc],
                        hTf_sb[:, t, :],
                        slab[:, vc * VC:(vc + 1) * VC],
                        start=False,
                        stop=(t == 7),
                    )
            for vc in range(NVC):
                nc.vector.tensor_copy(out_sb[:, vc * VC:(vc + 1) * VC], ops[vc])
            nc.sync.dma_start(out=out_l[:, :], in_=out_sb)

    nc.compile()
    return nc


def _get_nc():
    if "nc" not in _NC_CACHE:
        _NC_CACHE["nc"] = _build()
    return _NC_CACHE["nc"]


def _prep_in_maps(inputs):
    f32 = np.float32
    didx = np.asarray(inputs["decoding_input"]).astype(np.int32).reshape(B, 1)
    dh = np.asarray(inputs["decoder_hidden"], f32)[0]          # [64, 1024]
    encf = np.asarray(inputs["encoder_output"], f32)           # [1024, 64, 1024]
    embt = np.ascontiguousarray(np.asarray(inputs["emb_table"], f32))
    W_ih = np.asarray(inputs["W_ih"], f32)
    b_ih = np.asarray(inputs["b_ih"], f32)
    W_hh = np.asarray(inputs["W_hh"], f32)
    b_hh = np.asarray(inputs["b_hh"], f32)
    wen2 = np.ascontiguousarray(np.asarray(inputs["W_energy"], f32).reshape(2, H))
    benv = np.asarray(inputs["b_energy"], f32).reshape(1, 1)
    W_out = np.asarray(inputs["W_out"], f32)
    b_out = np.asarray(inputs["b_out"], f32)

    dhT = np.ascontiguousarray(dh.T)
    WihT = np.ascontiguousarray(W_ih.T)     # [2048, 3072]
    WhhT = np.ascontiguousarray(W_hh.T)     # [1024, 3072]
    WoutT = np.ascontiguousarray(W_out.T)   # [1024, 32000]

    in_maps = []
    for k in range(NC):
        bs = slice(k * BL, (k + 1) * BL)
        hs = slice(k * HL, (k + 1) * HL)
        gcols = np.concatenate(
            [np.arange(g * H + k * HL, g * H + k * HL + HL) for g in range(3)]
        )
        in_maps.append(
            {
                "enc": np.ascontiguousarray(encf[:, bs, :]),
                "dhsl": np.ascontiguousarray(dh[bs, :]),
                "dhT": dhT,
                "dhhs": np.ascontiguousarray(dh[:, hs]),
                "idx": np.ascontiguousarray(didx[bs, :]),
                "emb": embt,
                "wen": wen2,
                "ben": benv,
                "wiht": np.ascontiguousarray(WihT[:, gcols]),
                "whht": np.ascontiguousarray(WhhT[:, gcols]),
                "bih": np.ascontiguousarray(b_ih[gcols].reshape(1, -1)),
                "bhh": np.ascontiguousarray(b_hh[gcols].reshape(1, -1)),
                "woutt": np.ascontiguousarray(WoutT[:, k * VL:(k + 1) * VL]),
                "onesd": np.ones((1, B), f32),
                "onesr": np.ones((1, B), f32),
                "identr": np.eye(128, dtype=f32),
                "bout": np.ascontiguousarray(b_out[k * VL:(k + 1) * VL].reshape(1, -1)),
            }
        )

    return in_maps


def kernel(**inputs):
    global LAST_RESULTS
    nc = _get_nc()
    from concourse.bass_utils import run_bass_kernel_spmd

    in_maps = _prep_in_maps(inputs)
    res = run_bass_kernel_spmd(nc, in_maps, core_ids=list(range(NC)))
    LAST_RESULTS = res
    outs = res.results
    output = np.concatenate([outs[k]["out_l"] for k in range(NC)], axis=1)
    h_new = np.concatenate([outs[k]["hn_o"] for k in range(NC)], axis=1)[None]
    return output.astype(f32), h_new.astype(f32)
